# revision 17
# baseline (speedup 1.0000x reference)
"""GCMC GraphConv kernel for 8 Trainium2 NeuronCores.

Computation:  out = ci * segment_sum((input_feat @ weight * cj)[src], dst)

Strategy (dst-sharded, no collectives):
  - Nodes are 1D-partitioned by destination: core c owns dst rows
    [c*N/8, (c+1)*N/8).  Each edge is routed (on host) to the core owning
    its destination, so no cross-core reduction is needed.
  - Per core: h = (X @ W) * cj is computed for ALL nodes on the PE engine
    (X^T is replicated) and stored in HBM, split into 4 windows of 25000
    rows so gathers can use int16 indices and pipeline behind the matmul.
  - The per-edge gather h[src] uses the SWDGE dma_gather instruction
    (GPSIMD generates one 256B descriptor per edge).
  - The per-edge scatter-add over dst is done on the PE engine: edges are
    host-sorted by (dst block of 128, src window); for each 128-edge tile a
    one-hot matrix onehot[e, n] = (dst_local[e] == n) is built on the DVE
    (is_equal against an iota row) and matmul-accumulated into a PSUM tile
    for that dst block.  PSUM is flushed into an SBUF accumulator.
    This avoids any read-modify-write races that a DMA scatter-add with
    duplicate indices would have.
  - Finally acc is scaled by ci and written out; the host concatenates the
    8 core outputs.

The per-(window, block) tile count must be identical on all cores (single
SPMD program), so each group is padded to the max count over cores; pad
edges gather row 0 of the window and carry dst = -1 (never matches the
one-hot compare, so they contribute exactly zero).
"""

import dataclasses
import math

import numpy as np
import ml_dtypes

import concourse.bacc as bacc
import concourse.mybir as mybir
import concourse.tile as tile
from concourse.bass_utils import run_bass_kernel_spmd

BF16 = ml_dtypes.bfloat16
P = 128
NCORES = 8
D_IN = 128


@dataclasses.dataclass(frozen=True)
class Cfg:
    N: int = 100000
    D_OUT: int = 64          # 64 * 4B = 256B rows (dma_gather granularity)
    NWIN: int = 4            # src windows; N/NWIN must be < 32768 (int16 idx)
    SUPER: int = 2048        # phase-1 node supertile (cols of X^T per DMA)
    MAX_CHUNK_TILES: int = 4   # gather chunk; small so desc-gen never waits on
    #   ring drain (1024-desc calls measured 8.4ns/desc vs 1.6ns for 256)
    NQUEUES: int = 2         # SWDGE queues; alternate so desc-gen overlaps DMA
    SCRATCH: int = 32768     # dynamic DMA descriptor carveout (bytes/partition)
    H_BF16: bool = True      # compute h = X@W in bf16 (PE fp32 matmul is 4x slower)

    @property
    def n_loc(self):
        return self.N // NCORES

    @property
    def nblk(self):
        return math.ceil(self.n_loc / P)

    @property
    def win(self):
        return self.N // self.NWIN

    @property
    def tpw(self):
        return math.ceil(self.win / P)  # node tiles per window


CFG = Cfg()


# ---------------------------------------------------------------- host prep

def shard_edges(cfg: Cfg, src, dst):
    """Route and sort edges; build per-core padded index/dst arrays.

    Returns (G, per_core):
      G[w][b]     tiles of (window w, dst block b) — identical across cores
      per_core[c] dict with idx{w} [128, nw/16] int16 (wrapped+replicated)
                  and dstb{w} [128, nw/128] bf16 (edge j -> [j%128, j//128])
    """
    n_loc, nblk, win, nw_ = cfg.n_loc, cfg.nblk, cfg.win, cfg.NWIN
    src = np.asarray(src, dtype=np.int64)
    dst = np.asarray(dst, dtype=np.int64)
    core = dst // n_loc
    dst_loc = dst - core * n_loc
    blk = dst_loc >> 7
    dstb = (dst_loc & 127).astype(np.float32)
    wine = src // win
    src_loc = (src - wine * win).astype(np.int16)

    gid = (core * nw_ + wine) * nblk + blk
    counts = np.bincount(gid, minlength=NCORES * nw_ * nblk)
    counts = counts.reshape(NCORES, nw_, nblk)
    G = -(-counts.max(axis=0) // P)          # ceil tiles per (w, b)
    G[0] = np.maximum(G[0], 1)               # w=0 flush initializes acc
    tiles_w = G.sum(axis=1)                  # [NWIN]

    off_wb = np.zeros((nw_, nblk), dtype=np.int64)
    off_wb[:, 1:] = np.cumsum(G[:, :-1], axis=1) * P

    per_core = []
    for c in range(NCORES):
        m = core == c
        sl, db, we, bl = src_loc[m], dstb[m], wine[m], blk[m]
        key = we * nblk + bl
        o = np.argsort(key, kind="stable")
        ks = key[o]
        gcnt = np.bincount(ks, minlength=nw_ * nblk)
        gstart = np.concatenate([[0], np.cumsum(gcnt)[:-1]])
        within = np.arange(ks.size) - gstart[ks]
        wsel, bsel = ks // nblk, ks % nblk
        pos = off_wb[wsel, bsel] + within
        maps = {}
        for w in range(nw_):
            nw_edges = int(tiles_w[w]) * P
            ia = np.zeros(nw_edges, dtype=np.int16)        # pad -> row 0
            da = np.full(nw_edges, -1.0, dtype=np.float32)  # pad -> no match
            sel = wsel == w
            ia[pos[sel]] = sl[o][sel]
            da[pos[sel]] = db[o][sel]
            iw = np.ascontiguousarray(np.tile(ia.reshape(-1, 16).T, (8, 1)))
            dw = np.ascontiguousarray(da.reshape(-1, P).T.astype(BF16))
            maps[f"idx{w}"] = iw
            maps[f"dstb{w}"] = dw
        per_core.append(maps)
    return G, per_core


def host_inputs(cfg: Cfg, input_feat, weight, cj, ci):
    """Shared (replicated) device inputs + per-core civ."""
    N, dout, nblk, n_loc = cfg.N, cfg.D_OUT, cfg.nblk, cfg.n_loc
    mm_dt = BF16 if cfg.H_BF16 else np.float32
    xt = np.ascontiguousarray(np.asarray(input_feat, dtype=np.float32).T.astype(mm_dt))
    wgt = np.ascontiguousarray(np.asarray(weight, dtype=np.float32).astype(mm_dt))
    # cj laid out per (window, tile): col w*tpw+tt, partition = row in tile
    cjf = np.asarray(cj, dtype=np.float32).reshape(-1)
    ncols = cfg.NWIN * cfg.tpw
    cjv = np.zeros((P, ncols), dtype=np.float32)
    for w in range(cfg.NWIN):
        wvals = cjf[w * cfg.win:(w + 1) * cfg.win]
        pad = np.zeros(cfg.tpw * P, dtype=np.float32)
        pad[:wvals.size] = wvals
        cjv[:, w * cfg.tpw:(w + 1) * cfg.tpw] = pad.reshape(cfg.tpw, P).T
    iot = np.ascontiguousarray(
        np.broadcast_to(np.arange(P, dtype=np.float32), (P, P)).astype(BF16))
    cif = np.asarray(ci, dtype=np.float32).reshape(-1)
    civs = []
    for c in range(NCORES):
        cv = np.zeros(nblk * P, dtype=np.float32)
        cv[:n_loc] = cif[c * n_loc:(c + 1) * n_loc]
        civs.append(np.ascontiguousarray(cv.reshape(nblk, P).T))
    return {"xt": xt, "wgt": wgt, "cjv": cjv, "iot": iot}, civs


# ---------------------------------------------------------------- device IR

def tile_blocks(cfg: Cfg, G, w):
    """Per-tile (block, k, g) for window w, in edge order."""
    out = []
    for b in range(cfg.nblk):
        g = int(G[w][b])
        for k in range(g):
            out.append((b, k, g))
    return out


def build_nc(cfg: Cfg, G):
    f32, bf16, i16 = mybir.dt.float32, mybir.dt.bfloat16, mybir.dt.int16
    dout, nblk, win, tpw = cfg.D_OUT, cfg.nblk, cfg.win, cfg.tpw
    tiles_w = [int(sum(G[w])) for w in range(cfg.NWIN)]

    nc = bacc.Bacc("TRN2", target_bir_lowering=False, debug=False,
                   num_swdge_queues=cfg.NQUEUES,
                   dynamic_dma_scratch_size=cfg.SCRATCH)
    mm_dt = bf16 if cfg.H_BF16 else f32
    xt = nc.dram_tensor("xt", [D_IN, cfg.N], mm_dt, kind="ExternalInput")
    wgt = nc.dram_tensor("wgt", [D_IN, dout], mm_dt, kind="ExternalInput")
    cjv = nc.dram_tensor("cjv", [P, cfg.NWIN * tpw], f32, kind="ExternalInput")
    civ = nc.dram_tensor("civ", [P, nblk], f32, kind="ExternalInput")
    iot = nc.dram_tensor("iot", [P, P], bf16, kind="ExternalInput")
    idx_t = [nc.dram_tensor(f"idx{w}", [P, tiles_w[w] * 8], i16,
                            kind="ExternalInput") for w in range(cfg.NWIN)]
    dstb_t = [nc.dram_tensor(f"dstb{w}", [P, tiles_w[w]], bf16,
                             kind="ExternalInput") for w in range(cfg.NWIN)]
    hw_t = [nc.dram_tensor(f"h{w}", [tpw * P, dout], f32)
            for w in range(cfg.NWIN)]
    out_t = nc.dram_tensor("out", [nblk * P, dout], f32, kind="ExternalOutput")

    with tile.TileContext(nc) as tc:
        with (
            tc.tile_pool(name="const", bufs=1) as cpool,
            tc.tile_pool(name="xt", bufs=3) as xpool,
            tc.tile_pool(name="hs", bufs=3) as hpool,
            tc.tile_pool(name="idx", bufs=2) as ipool,
            tc.tile_pool(name="msg", bufs=2) as mpool,
            tc.tile_pool(name="oh", bufs=6) as opool,
            tc.tile_pool(name="ps", bufs=8, space="PSUM") as pspool,
            tc.tile_pool(name="acc", bufs=1) as apool,
        ):
            wgt_sb = cpool.tile([P, dout], mm_dt, tag="wgt")
            nc.sync.dma_start(out=wgt_sb[:], in_=wgt[:])
            iota_sb = cpool.tile([P, P], bf16, tag="iot")
            nc.sync.dma_start(out=iota_sb[:], in_=iot[:])
            cj_sb = cpool.tile([P, cfg.NWIN * tpw], f32, tag="cj")
            nc.sync.dma_start(out=cj_sb[:], in_=cjv[:])
            ci_sb = cpool.tile([P, nblk], f32, tag="ci")
            nc.sync.dma_start(out=ci_sb[:], in_=civ[:])
            acc = apool.tile([P, nblk * dout], f32, tag="acc")

            for w in range(cfg.NWIN):
                # ---- phase 1: h_w = (X @ W) * cj for window w ----
                for s in range(0, win, cfg.SUPER):
                    ncols = min(cfg.SUPER, win - s)
                    nsub = math.ceil(ncols / P)
                    xt_sb = xpool.tile([P, cfg.SUPER], mm_dt, tag="xt")
                    nc.sync.dma_start(out=xt_sb[:, :ncols],
                                      in_=xt[:, w * win + s:w * win + s + ncols])
                    hs_sb = hpool.tile([P, (cfg.SUPER // P) * dout], f32, tag="hs")
                    for q in range(nsub):
                        rows = min(P, ncols - q * P)
                        ps = pspool.tile([P, dout], f32, tag="ps")
                        nc.tensor.matmul(
                            out=ps[:rows],
                            lhsT=xt_sb[:, q * P:q * P + rows],
                            rhs=wgt_sb[:],
                            start=True, stop=True)
                        tt = (s + q * P) // P  # tile index within window
                        nc.scalar.mul(
                            hs_sb[:rows, q * dout:(q + 1) * dout],
                            ps[:rows],
                            cj_sb[:rows, w * tpw + tt:w * tpw + tt + 1])
                    nfull = ncols // P  # full 128-row subtiles
                    if nfull:
                        nc.sync.dma_start(
                            out=hw_t[w][s:s + nfull * P, :].rearrange(
                                "(q p) f -> p q f", p=P),
                            in_=hs_sb[:, :nfull * dout].rearrange(
                                "p (q f) -> p q f", f=dout))
                    if nsub > nfull:  # ragged tail: only the valid rows
                        rows = ncols - nfull * P
                        nc.sync.dma_start(
                            out=hw_t[w][s + nfull * P:s + ncols, :],
                            in_=hs_sb[:rows, nfull * dout:nsub * dout])

                # ---- phase 2: gather + one-hot scatter for window w ----
                idx_sb = ipool.tile([P, tiles_w[w] * 8], i16, tag="idx")
                nc.sync.dma_start(out=idx_sb[:], in_=idx_t[w][:])
                dst_sb = ipool.tile([P, tiles_w[w]], bf16, tag="dstb")
                nc.sync.dma_start(out=dst_sb[:], in_=dstb_t[w][:])

                tb = tile_blocks(cfg, G, w)
                ps = None
                for t0 in range(0, len(tb), cfg.MAX_CHUNK_TILES):
                    t1 = min(t0 + cfg.MAX_CHUNK_TILES, len(tb))
                    nt = t1 - t0
                    ne = nt * P
                    msg = mpool.tile([P, nt * dout], f32, tag="msg")
                    nc.gpsimd.dma_gather(
                        msg[:].rearrange("p (t f) -> p t f", f=dout),
                        hw_t[w][:],
                        idx_sb[:, t0 * 8:t1 * 8],
                        ne, ne, dout,
                        queue_num=(t0 // cfg.MAX_CHUNK_TILES) % cfg.NQUEUES)
                    msg16 = mpool.tile([P, nt * dout], bf16, tag="msg16")
                    nc.scalar.copy(msg16[:], msg[:])
                    for t in range(t0, t1):
                        b, k, g = tb[t]
                        if k == 0:
                            ps = pspool.tile([P, dout], f32, tag="ps")
                        oh = opool.tile([P, P], bf16, tag="oh")
                        nc.vector.tensor_scalar(
                            out=oh[:],
                            in0=iota_sb[:],
                            scalar1=dst_sb[:, t:t + 1],
                            scalar2=None,
                            op0=mybir.AluOpType.is_equal)
                        nc.tensor.matmul(
                            out=ps[:],
                            lhsT=oh[:],
                            rhs=msg16[:, (t - t0) * dout:(t - t0 + 1) * dout],
                            start=(k == 0), stop=(k == g - 1))
                        if k == g - 1:
                            if w == 0:
                                nc.vector.tensor_copy(
                                    out=acc[:, b * dout:(b + 1) * dout],
                                    in_=ps[:])
                            else:
                                nc.vector.tensor_add(
                                    out=acc[:, b * dout:(b + 1) * dout],
                                    in0=acc[:, b * dout:(b + 1) * dout],
                                    in1=ps[:])

            # ---- epilogue: scale by ci, store ----
            for b in range(nblk):
                nc.vector.tensor_mul(
                    out=acc[:, b * dout:(b + 1) * dout],
                    in0=acc[:, b * dout:(b + 1) * dout],
                    in1=ci_sb[:, b:b + 1].to_broadcast([P, dout]))
            nc.sync.dma_start(
                out=out_t[:].rearrange("(b p) f -> p b f", p=P),
                in_=acc[:].rearrange("p (b f) -> p b f", f=dout))
    nc.compile()
    return nc


# ---------------------------------------------------------------- entry

def run(cfg: Cfg, input_feat, weight, cj, ci, src_idx, dst_idx, **run_kwargs):
    G, per_core = shard_edges(cfg, src_idx, dst_idx)
    shared, civs = host_inputs(cfg, input_feat, weight, cj, ci)
    nc = build_nc(cfg, G)
    in_maps = []
    for c in range(NCORES):
        m = dict(shared)
        m["civ"] = civs[c]
        m.update(per_core[c])
        in_maps.append(m)
    res = run_bass_kernel_spmd(nc, in_maps, core_ids=list(range(NCORES)),
                               **run_kwargs)
    outs = [res.results[c]["out"][:cfg.n_loc] for c in range(NCORES)]
    full = np.concatenate(outs, axis=0).astype(np.float32)
    return full, res


def kernel(input_feat, weight, cj, ci, src_idx, dst_idx):
    out, _ = run(CFG, input_feat, weight, cj, ci, src_idx, dst_idx)
    return out


# revision 23
# speedup vs baseline: 1.0949x; 1.0949x over previous
"""GCMC GraphConv kernel for 8 Trainium2 NeuronCores.

Computation:  out = ci * segment_sum((input_feat @ weight * cj)[src], dst)

Strategy (dst-sharded, no collectives):
  - Nodes are 1D-partitioned by destination: core c owns dst rows
    [c*N/8, (c+1)*N/8).  Each edge is routed (on host) to the core owning
    its destination, so no cross-core reduction is needed.
  - Per core: h = (X @ W) * cj is computed for ALL nodes on the PE engine
    (X^T is replicated) and stored in HBM, split into 4 windows of 25000
    rows so gathers can use int16 indices and pipeline behind the matmul.
  - The per-edge gather h[src] uses the SWDGE dma_gather instruction
    (GPSIMD generates one 256B descriptor per edge).
  - The per-edge scatter-add over dst is done on the PE engine: edges are
    host-sorted by (dst block of 128, src window); for each 128-edge tile a
    one-hot matrix onehot[e, n] = (dst_local[e] == n) is built on the DVE
    (is_equal against an iota row) and matmul-accumulated into a PSUM tile
    for that dst block.  PSUM is flushed into an SBUF accumulator.
    This avoids any read-modify-write races that a DMA scatter-add with
    duplicate indices would have.
  - Finally acc is scaled by ci and written out; the host concatenates the
    8 core outputs.

The per-(window, block) tile count must be identical on all cores (single
SPMD program), so each group is padded to the max count over cores; pad
edges gather row 0 of the window and carry dst = -1 (never matches the
one-hot compare, so they contribute exactly zero).
"""

import dataclasses
import math

import numpy as np
import ml_dtypes

import concourse.bacc as bacc
import concourse.mybir as mybir
import concourse.tile as tile
from concourse.bass_utils import run_bass_kernel_spmd

BF16 = ml_dtypes.bfloat16
P = 128
NCORES = 8
D_IN = 128


@dataclasses.dataclass(frozen=True)
class Cfg:
    N: int = 100000
    D_OUT: int = 64          # 64 * 4B = 256B rows (dma_gather granularity)
    NWIN: int = 4            # src windows; N/NWIN must be < 32768 (int16 idx)
    SUPER: int = 2048        # phase-1 node supertile (cols of X^T per DMA)
    MAX_CHUNK_TILES: int = 4   # gather chunk; small so desc-gen never waits on
    #   ring drain (1024-desc calls measured 8.4ns/desc vs 1.6ns for 256)
    NQUEUES: int = 4         # SWDGE queues; alternate so desc-gen overlaps DMA
    SCRATCH: int = 32768     # dynamic DMA descriptor carveout (bytes/partition)
    H_BF16: bool = True      # compute h = X@W in bf16 (PE fp32 matmul is 4x slower)

    @property
    def n_loc(self):
        return self.N // NCORES

    @property
    def nblk(self):
        return math.ceil(self.n_loc / P)

    @property
    def win(self):
        return self.N // self.NWIN

    @property
    def tpw(self):
        return math.ceil(self.win / P)  # node tiles per window


CFG = Cfg()


# ---------------------------------------------------------------- host prep

def shard_edges(cfg: Cfg, src, dst):
    """Route and sort edges; build per-core padded index/dst arrays.

    Returns (G, per_core):
      G[w][b]     tiles of (window w, dst block b) — identical across cores
      per_core[c] dict with idx{w} [128, nw/16] int16 (wrapped+replicated)
                  and dstb{w} [128, nw/128] bf16 (edge j -> [j%128, j//128])
    """
    n_loc, nblk, win, nw_ = cfg.n_loc, cfg.nblk, cfg.win, cfg.NWIN
    src = np.asarray(src, dtype=np.int64)
    dst = np.asarray(dst, dtype=np.int64)
    core = dst // n_loc
    dst_loc = dst - core * n_loc
    blk = dst_loc >> 7
    dstb = (dst_loc & 127).astype(np.float32)
    wine = src // win
    src_loc = (src - wine * win).astype(np.int16)

    gid = (core * nw_ + wine) * nblk + blk
    counts = np.bincount(gid, minlength=NCORES * nw_ * nblk)
    counts = counts.reshape(NCORES, nw_, nblk)
    G = -(-counts.max(axis=0) // P)          # ceil tiles per (w, b)
    G[0] = np.maximum(G[0], 1)               # w=0 flush initializes acc
    tiles_w = G.sum(axis=1)                  # [NWIN]

    off_wb = np.zeros((nw_, nblk), dtype=np.int64)
    off_wb[:, 1:] = np.cumsum(G[:, :-1], axis=1) * P

    per_core = []
    for c in range(NCORES):
        m = core == c
        sl, db, we, bl = src_loc[m], dstb[m], wine[m], blk[m]
        key = we * nblk + bl
        o = np.argsort(key, kind="stable")
        ks = key[o]
        gcnt = np.bincount(ks, minlength=nw_ * nblk)
        gstart = np.concatenate([[0], np.cumsum(gcnt)[:-1]])
        within = np.arange(ks.size) - gstart[ks]
        wsel, bsel = ks // nblk, ks % nblk
        pos = off_wb[wsel, bsel] + within
        maps = {}
        for w in range(nw_):
            nw_edges = int(tiles_w[w]) * P
            ia = np.zeros(nw_edges, dtype=np.int16)        # pad -> row 0
            da = np.full(nw_edges, -1.0, dtype=np.float32)  # pad -> no match
            sel = wsel == w
            ia[pos[sel]] = sl[o][sel]
            da[pos[sel]] = db[o][sel]
            iw = np.ascontiguousarray(np.tile(ia.reshape(-1, 16).T, (8, 1)))
            dw = np.ascontiguousarray(da.reshape(-1, P).T)
            maps[f"idx{w}"] = iw
            maps[f"dstb{w}"] = dw
        per_core.append(maps)
    return G, per_core


def host_inputs(cfg: Cfg, input_feat, weight, cj, ci):
    """Shared (replicated) device inputs + per-core civ."""
    N, dout, nblk, n_loc = cfg.N, cfg.D_OUT, cfg.nblk, cfg.n_loc
    mm_dt = BF16 if cfg.H_BF16 else np.float32
    xt = np.ascontiguousarray(np.asarray(input_feat, dtype=np.float32).T.astype(mm_dt))
    wgt = np.ascontiguousarray(np.asarray(weight, dtype=np.float32).astype(mm_dt))
    # cj laid out per (window, tile): col w*tpw+tt, partition = row in tile
    cjf = np.asarray(cj, dtype=np.float32).reshape(-1)
    ncols = cfg.NWIN * cfg.tpw
    cjv = np.zeros((P, ncols), dtype=np.float32)
    for w in range(cfg.NWIN):
        wvals = cjf[w * cfg.win:(w + 1) * cfg.win]
        pad = np.zeros(cfg.tpw * P, dtype=np.float32)
        pad[:wvals.size] = wvals
        cjv[:, w * cfg.tpw:(w + 1) * cfg.tpw] = pad.reshape(cfg.tpw, P).T
    iot = np.ascontiguousarray(
        np.broadcast_to(np.arange(P, dtype=np.float32), (P, P)).astype(BF16))
    cif = np.asarray(ci, dtype=np.float32).reshape(-1)
    civs = []
    for c in range(NCORES):
        cv = np.zeros(nblk * P, dtype=np.float32)
        cv[:n_loc] = cif[c * n_loc:(c + 1) * n_loc]
        civs.append(np.ascontiguousarray(cv.reshape(nblk, P).T))
    return {"xt": xt, "wgt": wgt, "cjv": cjv, "iot": iot}, civs


# ---------------------------------------------------------------- device IR

def tile_blocks(cfg: Cfg, G, w):
    """Per-tile (block, k, g) for window w, in edge order."""
    out = []
    for b in range(cfg.nblk):
        g = int(G[w][b])
        for k in range(g):
            out.append((b, k, g))
    return out


def build_nc(cfg: Cfg, G):
    f32, bf16, i16 = mybir.dt.float32, mybir.dt.bfloat16, mybir.dt.int16
    dout, nblk, win, tpw = cfg.D_OUT, cfg.nblk, cfg.win, cfg.tpw
    tiles_w = [int(sum(G[w])) for w in range(cfg.NWIN)]

    nc = bacc.Bacc("TRN2", target_bir_lowering=False, debug=False,
                   num_swdge_queues=cfg.NQUEUES,
                   dynamic_dma_scratch_size=cfg.SCRATCH)
    mm_dt = bf16 if cfg.H_BF16 else f32
    xt = nc.dram_tensor("xt", [D_IN, cfg.N], mm_dt, kind="ExternalInput")
    wgt = nc.dram_tensor("wgt", [D_IN, dout], mm_dt, kind="ExternalInput")
    cjv = nc.dram_tensor("cjv", [P, cfg.NWIN * tpw], f32, kind="ExternalInput")
    civ = nc.dram_tensor("civ", [P, nblk], f32, kind="ExternalInput")
    iot = nc.dram_tensor("iot", [P, P], bf16, kind="ExternalInput")
    idx_t = [nc.dram_tensor(f"idx{w}", [P, tiles_w[w] * 8], i16,
                            kind="ExternalInput") for w in range(cfg.NWIN)]
    dstb_t = [nc.dram_tensor(f"dstb{w}", [P, tiles_w[w]], f32,
                             kind="ExternalInput") for w in range(cfg.NWIN)]
    hw_t = [nc.dram_tensor(f"h{w}", [tpw * P, dout], f32)
            for w in range(cfg.NWIN)]
    out_t = nc.dram_tensor("out", [nblk * P, dout], f32, kind="ExternalOutput")

    with tile.TileContext(nc) as tc:
        with (
            tc.tile_pool(name="const", bufs=1) as cpool,
            tc.tile_pool(name="xt", bufs=3) as xpool,
            tc.tile_pool(name="hs", bufs=3) as hpool,
            tc.tile_pool(name="idx", bufs=2) as ipool,
            tc.tile_pool(name="msg", bufs=2) as mpool,
            tc.tile_pool(name="oh", bufs=6) as opool,
            tc.tile_pool(name="ps", bufs=4, space="PSUM") as pspool,
            tc.tile_pool(name="acc", bufs=1) as apool,
        ):
            wgt_sb = cpool.tile([P, dout], mm_dt, tag="wgt")
            nc.sync.dma_start(out=wgt_sb[:], in_=wgt[:])
            iota_sb = cpool.tile([P, P], bf16, tag="iot")
            nc.sync.dma_start(out=iota_sb[:], in_=iot[:])
            cj_sb = cpool.tile([P, cfg.NWIN * tpw], f32, tag="cj")
            nc.sync.dma_start(out=cj_sb[:], in_=cjv[:])
            ci_sb = cpool.tile([P, nblk], f32, tag="ci")
            nc.sync.dma_start(out=ci_sb[:], in_=civ[:])
            acc = apool.tile([P, nblk * dout], f32, tag="acc")

            def emit_super(w, s):
                """Phase 1: one supertile of h_w = (X @ W) * cj."""
                ncols = min(cfg.SUPER, win - s)
                nsub = math.ceil(ncols / P)
                xt_sb = xpool.tile([P, cfg.SUPER], mm_dt, tag="xt")
                nc.sync.dma_start(out=xt_sb[:, :ncols],
                                  in_=xt[:, w * win + s:w * win + s + ncols])
                hs_sb = hpool.tile([P, (cfg.SUPER // P) * dout], f32, tag="hs")
                for q in range(nsub):
                    rows = min(P, ncols - q * P)
                    ps = pspool.tile([P, dout], f32, tag="ps1")
                    nc.tensor.matmul(
                        out=ps[:rows],
                        lhsT=xt_sb[:, q * P:q * P + rows],
                        rhs=wgt_sb[:],
                        start=True, stop=True)
                    tt = (s + q * P) // P  # tile index within window
                    nc.scalar.mul(
                        hs_sb[:rows, q * dout:(q + 1) * dout],
                        ps[:rows],
                        cj_sb[:rows, w * tpw + tt:w * tpw + tt + 1])
                nfull = ncols // P  # full 128-row subtiles
                if nfull:
                    nc.sync.dma_start(
                        out=hw_t[w][s:s + nfull * P, :].rearrange(
                            "(q p) f -> p q f", p=P),
                        in_=hs_sb[:, :nfull * dout].rearrange(
                            "p (q f) -> p q f", f=dout))
                if nsub > nfull:  # ragged tail: only the valid rows
                    rows = ncols - nfull * P
                    nc.sync.dma_start(
                        out=hw_t[w][s + nfull * P:s + ncols, :],
                        in_=hs_sb[:rows, nfull * dout:nsub * dout])

            st = {"ps": None}

            def emit_chunk(w, t0, tb, idx_sb, dst_sb, qn):
                """Phase 2: gather one chunk of edges and matmul-scatter it."""
                t1 = min(t0 + cfg.MAX_CHUNK_TILES, len(tb))
                nt = t1 - t0
                ne = nt * P
                msg = mpool.tile([P, nt * dout], f32, tag="msg")
                nc.gpsimd.dma_gather(
                    msg[:].rearrange("p (t f) -> p t f", f=dout),
                    hw_t[w][:],
                    idx_sb[:, t0 * 8:t1 * 8],
                    ne, ne, dout,
                    queue_num=qn)
                msg16 = mpool.tile([P, nt * dout], bf16, tag="msg16")
                nc.scalar.copy(msg16[:], msg[:])
                for t in range(t0, t1):
                    b, k, g = tb[t]
                    if k == 0:
                        st["ps"] = pspool.tile([P, dout], f32, tag="ps2",
                                               name="ps2")
                    ps = st["ps"]
                    oh = opool.tile([P, P], bf16, tag="oh")
                    nc.vector.tensor_scalar(
                        out=oh[:],
                        in0=iota_sb[:],
                        scalar1=dst_sb[:, t:t + 1],
                        scalar2=None,
                        op0=mybir.AluOpType.is_equal)
                    nc.tensor.matmul(
                        out=ps[:],
                        lhsT=oh[:],
                        rhs=msg16[:, (t - t0) * dout:(t - t0 + 1) * dout],
                        start=(k == 0), stop=(k == g - 1))
                    if k == g - 1:
                        if w == 0:
                            nc.vector.tensor_copy(
                                out=acc[:, b * dout:(b + 1) * dout], in_=ps[:])
                        else:
                            nc.vector.tensor_add(
                                out=acc[:, b * dout:(b + 1) * dout],
                                in0=acc[:, b * dout:(b + 1) * dout],
                                in1=ps[:])

            super_starts = list(range(0, win, cfg.SUPER))
            for s in super_starts:
                emit_super(0, s)
            qn = 0
            for w in range(cfg.NWIN):
                idx_sb = ipool.tile([P, tiles_w[w] * 8], i16, tag="idx")
                nc.sync.dma_start(out=idx_sb[:], in_=idx_t[w][:])
                dst_sb = ipool.tile([P, tiles_w[w]], f32, tag="dstb")
                nc.sync.dma_start(out=dst_sb[:], in_=dstb_t[w][:])

                tb = tile_blocks(cfg, G, w)
                chunk_starts = list(range(0, len(tb), cfg.MAX_CHUNK_TILES))
                # software-pipeline: weave window w+1's phase 1 between
                # window w's gather/scatter chunks so PE/ACT/DMA stay busy
                # while the gather queue drains
                nxt = super_starts if w + 1 < cfg.NWIN else []
                frac, si = 0.0, 0
                ratio = len(nxt) / max(1, len(chunk_starts))
                for t0 in chunk_starts:
                    emit_chunk(w, t0, tb, idx_sb, dst_sb, qn)
                    qn = (qn + 1) % cfg.NQUEUES
                    frac += ratio
                    while frac >= 1.0 and si < len(nxt):
                        emit_super(w + 1, nxt[si])
                        si += 1
                        frac -= 1.0
                while si < len(nxt):
                    emit_super(w + 1, nxt[si])
                    si += 1

            # ---- epilogue: scale by ci, store ----
            for b in range(nblk):
                nc.vector.tensor_mul(
                    out=acc[:, b * dout:(b + 1) * dout],
                    in0=acc[:, b * dout:(b + 1) * dout],
                    in1=ci_sb[:, b:b + 1].to_broadcast([P, dout]))
            nc.sync.dma_start(
                out=out_t[:].rearrange("(b p) f -> p b f", p=P),
                in_=acc[:].rearrange("p (b f) -> p b f", f=dout))
    nc.compile()
    return nc


# ---------------------------------------------------------------- entry

def run(cfg: Cfg, input_feat, weight, cj, ci, src_idx, dst_idx, **run_kwargs):
    G, per_core = shard_edges(cfg, src_idx, dst_idx)
    shared, civs = host_inputs(cfg, input_feat, weight, cj, ci)
    nc = build_nc(cfg, G)
    in_maps = []
    for c in range(NCORES):
        m = dict(shared)
        m["civ"] = civs[c]
        m.update(per_core[c])
        in_maps.append(m)
    res = run_bass_kernel_spmd(nc, in_maps, core_ids=list(range(NCORES)),
                               **run_kwargs)
    outs = [res.results[c]["out"][:cfg.n_loc] for c in range(NCORES)]
    full = np.concatenate(outs, axis=0).astype(np.float32)
    return full, res


def kernel(input_feat, weight, cj, ci, src_idx, dst_idx):
    out, _ = run(CFG, input_feat, weight, cj, ci, src_idx, dst_idx)
    return out
